# revision 26
# baseline (speedup 1.0000x reference)
import sys

for _p in ("/opt/trn_rl_repo",):
    if _p not in sys.path:
        sys.path.insert(0, _p)

import numpy as np
import ml_dtypes

import concourse.bass as bass
import concourse.mybir as mybir
import concourse.tile as tile
from concourse import bacc
from concourse.bass_types import AP
from concourse.bass_utils import run_bass_kernel_spmd

V, L, H, DH, D, DI = 50257, 6, 8, 64, 512, 2048
QLEN, MLEN, BSZ = 512, 512, 4
KLEN = QLEN + MLEN
NCORES = 8
KS = D // 128            # 4 contraction sub-tiles
DIT = DI // 128          # 16
TM = QLEN // 128         # 4 query/token tiles
NJ = KLEN // 128         # 8 key sub-tiles
VSPLIT = 25128           # vocab half boundary: [0,25128) / [25128,50257)
VH = 25600               # padded per-half vocab (50 chunks of 512)
NCH = VH // 512          # 50
PADS = (VH - VSPLIT, VH - (V - VSPLIT))   # exp(0)=1 contributions to subtract
SCALE = 1.0 / (DH ** 0.5)
MASKVAL = -30000.0
EPS = 1e-5

bf16 = mybir.dt.bfloat16
f32 = mybir.dt.float32
FT = mybir.ActivationFunctionType
OP = mybir.AluOpType

_NC_CACHE = {}
LAST_RESULTS = None


def build_nc(niter=1, sim=False):
    key = ("nc", niter, sim)
    if key in _NC_CACHE:
        return _NC_CACHE[key]
    # Bacc (not plain Bass): its compile() runs generate_event_semaphores,
    # which splits multi-sem waits to satisfy the 1-wait-per-instruction HW
    # constraint.
    nc = bacc.Bacc("TRN2", target_bir_lowering=False, debug=sim)
    h0t = nc.dram_tensor("h0t", [128, KS, QLEN], bf16, kind="ExternalInput")
    memst = nc.dram_tensor("memst", [L, 128, KS, MLEN], bf16, kind="ExternalInput")
    qkvwt = nc.dram_tensor("qkvwt", [L, 128, KS, 3 * H * DH], bf16, kind="ExternalInput")
    rwt = nc.dram_tensor("rwt", [L, 128, KS, H * DH], bf16, kind="ExternalInput")
    owt = nc.dram_tensor("owt", [L, 128, KS, D], bf16, kind="ExternalInput")
    w1t = nc.dram_tensor("w1t", [L, 128, KS, DI], bf16, kind="ExternalInput")
    w2t = nc.dram_tensor("w2t", [L, 128, DIT, D], bf16, kind="ExternalInput")
    rt_d = nc.dram_tensor("rt", [128, KS, KLEN], bf16, kind="ExternalInput")
    rbias_d = nc.dram_tensor("rbias", [128, 2 * KS], f32, kind="ExternalInput")
    b1t = nc.dram_tensor("b1t", [L, 128, DIT], f32, kind="ExternalInput")
    wtgt_d = nc.dram_tensor("wtgt", [128, KS, QLEN], bf16, kind="ExternalInput")
    owv_d = nc.dram_tensor("owv", [128, KS, VH], bf16, kind="ExternalInput")
    sums_o = nc.dram_tensor("sums", [128, TM], f32, kind="ExternalOutput")
    tl_o = nc.dram_tensor("tl", [1, QLEN], f32, kind="ExternalOutput")

    NBD = 2
    with tile.TileContext(nc) as tc:
        with (
            tc.tile_pool(name="singles", bufs=1) as singles,
            tc.tile_pool(name="wq", bufs=1) as wq,
            tc.tile_pool(name="wf", bufs=1) as wf,
            tc.tile_pool(name="act", bufs=1) as apool,
            tc.tile_pool(name="hh", bufs=3) as hpool,
            tc.tile_pool(name="hs", bufs=2) as hspool,
            tc.tile_pool(name="sc", bufs=2) as scpool,
            tc.tile_pool(name="et", bufs=4) as etpool,
            tc.tile_pool(name="rows", bufs=8) as rows,
            tc.tile_pool(name="bcast", bufs=2) as bcast,
            tc.tile_pool(name="dens", bufs=4) as dens,
            tc.tile_pool(name="lse", bufs=2) as lsep,
            tc.tile_pool(name="ps", bufs=2, space="PSUM") as ps,
            tc.tile_pool(name="pav", bufs=2, space="PSUM") as pav,
            tc.tile_pool(name="pst", bufs=2, space="PSUM") as pst,
        ):
            # loop-invariant loads.  The HW allows only ~3 semaphore waits
            # per instruction, so absorb the DMA-lane waits into one sync-
            # engine NOP (3 waits), run the memsets after it, then raise a
            # barrier whose own wait set is tiny.  Downstream instructions
            # then wait on the barrier only.
            from concourse.tile_rust import add_dep_helper
            rt_sb = singles.tile([128, KS, KLEN], bf16, tag="rt")
            d1 = nc.sync.dma_start(rt_sb[:], rt_d[:])
            rbias = singles.tile([128, 2 * KS], f32, tag="rbias")
            d2 = nc.sync.dma_start(rbias[:], rbias_d[:])
            rwb = rbias[:, 0:KS]
            rrb = rbias[:, KS:2 * KS]
            wtgt_sb = singles.tile([128, KS, QLEN], bf16, tag="wtgt")
            d3 = nc.sync.dma_start(wtgt_sb[:], wtgt_d[:])
            n1 = nc.sync.nop()
            for d_ in (d1, d2):
                add_dep_helper(n1.ins, d_.ins, sync=True, reason="absorb dma lanes")
            n2 = nc.sync.nop()
            add_dep_helper(n2.ins, d3.ins, sync=True, reason="absorb dma lanes")
            ones_sb = singles.tile([128, 1], bf16, tag="ones")
            nc.gpsimd.memset(ones_sb[:], 1.0)
            # rel-shift shear sources: upper halves hold the causal-mask fill
            # so the shear read applies shift+mask in one DMA.  Allocated once
            # (persistent tiles) so the fill survives all iterations.
            bd_raws = [singles.tile([128, 2 * KLEN], bf16, tag=f"bdraw{i}", name=f"bdraw{i}")
                       for i in range(NBD)]
            for t_ in bd_raws:
                nc.gpsimd.memset(t_[:, KLEN:], MASKVAL)
            tc.strict_bb_all_engine_barrier()

            bd_state = [0]

            def mm(out, lhsT, rhs, start, stop):
                nc.tensor.matmul(out, lhsT, rhs, start=start, stop=stop)

            def layer(l, h_in):
                ldmas = []
                qkvw = wq.tile([128, KS, 3 * H * DH], bf16, tag="qkvw")
                ldmas.append(nc.sync.dma_start(qkvw[:], qkvwt[l]))
                rw = wq.tile([128, KS, H * DH], bf16, tag="rw")
                ldmas.append(nc.sync.dma_start(rw[:], rwt[l]))
                ow = wq.tile([128, KS, D], bf16, tag="ow")
                ldmas.append(nc.sync.dma_start(ow[:], owt[l]))
                w1 = wf.tile([128, KS, DI], bf16, tag="w1")
                ldmas.append(nc.sync.dma_start(w1[:], w1t[l]))
                w2 = wf.tile([128, DIT, D], bf16, tag="w2")
                ldmas.append(nc.sync.dma_start(w2[:], w2t[l]))
                b1 = dens.tile([128, DIT], f32, tag="b1")
                ldmas.append(nc.sync.dma_start(b1[:], b1t[l]))

                cat = apool.tile([128, KS, KLEN], bf16, tag="cat")
                ldmas.append(nc.sync.dma_start(cat[:, :, 0:MLEN], memst[l]))
                nc.vector.tensor_copy(cat[:, :, MLEN:KLEN], h_in[:])

                # ---- projections ----
                q_rw = apool.tile([128, KS, QLEN], bf16, tag="q_rw")
                q_rr = apool.tile([128, KS, QLEN], bf16, tag="q_rr")
                for m in range(KS):  # q feature tiles
                    qp = ps.tile([128, KLEN], f32, tag="sc")
                    for k in range(KS):
                        mm(qp[:, 0:QLEN], qkvw[:, k, 128 * m:128 * m + 128],
                           cat[:, k, MLEN:KLEN], k == 0, k == KS - 1)
                    nc.scalar.activation(q_rw[:, m, :], qp[:, 0:QLEN], FT.Identity,
                                         bias=rwb[:, m:m+1], scale=1.0)
                    nc.scalar.activation(q_rr[:, m, :], qp[:, 0:QLEN], FT.Identity,
                                         bias=rrb[:, m:m+1], scale=1.0)

                k_sb = apool.tile([128, KS, KLEN], bf16, tag="k_sb")
                for m in range(KS):  # k feature tiles
                    kp = ps.tile([128, KLEN], f32, tag="sc")
                    for half in range(2):
                        for k in range(KS):
                            mm(kp[:, 512 * half:512 * half + 512],
                               qkvw[:, k, 512 + 128 * m:512 + 128 * m + 128],
                               cat[:, k, 512 * half:512 * half + 512], k == 0, k == KS - 1)
                    nc.scalar.copy(k_sb[:, m, :], kp[:])

                v_sb = apool.tile([128, NJ, 512], bf16, tag="v_sb")  # token-major
                for js in range(NJ):
                    vp = ps.tile([128, KLEN], f32, tag="sc")
                    for k in range(KS):
                        mm(vp[:, 0:512], cat[:, k, 128 * js:128 * js + 128],
                           qkvw[:, k, 1024:1536], k == 0, k == KS - 1)
                    nc.scalar.copy(v_sb[:, js, :], vp[:, 0:512])

                rk_sb = apool.tile([128, KS, KLEN], bf16, tag="rk_sb")
                for m in range(KS):
                    rp = ps.tile([128, KLEN], f32, tag="sc")
                    for half in range(2):
                        for k in range(KS):
                            mm(rp[:, 512 * half:512 * half + 512],
                               rw[:, k, 128 * m:128 * m + 128],
                               rt_sb[:, k, 512 * half:512 * half + 512], k == 0, k == KS - 1)
                    nc.scalar.copy(rk_sb[:, m, :], rp[:])

                # absorb this layer's weight-DMA lane waits into sync-engine
                # NOPs (<=3 sem waits each) so the first shear/transpose DMAs
                # of the attention loop stay under the HW wait-slot limit.
                for i0 in range(0, len(ldmas), 2):
                    n_ = nc.sync.nop()
                    for d_ in ldmas[i0:i0 + 2]:
                        add_dep_helper(n_.ins, d_.ins, sync=True,
                                       reason="absorb layer dma lanes")

                # ---- attention ----
                vecT = apool.tile([128, KS, QLEN], bf16, tag="vecT")
                for h in range(H):
                    t = h // 2
                    po = 64 * (h % 2)
                    et_tiles = []
                    for qi in range(TM):
                        bdp = ps.tile([128, KLEN], f32, tag="sc")
                        for half in range(2):
                            mm(bdp[:, 512 * half:512 * half + 512],
                               q_rr[po:po + 64, t, 128 * qi:128 * qi + 128],
                               rk_sb[po:po + 64, t, 512 * half:512 * half + 512],
                               True, True)
                        bd = bd_raws[bd_state[0] % NBD]
                        bd_state[0] += 1
                        nc.vector.tensor_copy(bd[:, 0:KLEN], bdp[:])
                        bdsh = hspool.tile([128, KLEN], bf16, tag="bdsh")
                        bap = bd[:]
                        shear = AP(bap.tensor, bap.offset + 511 - 128 * qi,
                                   [[2 * KLEN - 1, 128], [1, KLEN]])
                        nc.sync.dma_start(bdsh[:], shear)
                        acp = ps.tile([128, KLEN], f32, tag="sc")
                        for half in range(2):
                            mm(acp[:, 512 * half:512 * half + 512],
                               q_rw[po:po + 64, t, 128 * qi:128 * qi + 128],
                               k_sb[po:po + 64, t, 512 * half:512 * half + 512],
                               True, True)
                        s_sb = hspool.tile([128, KLEN], bf16, tag="s_sb")
                        nc.vector.tensor_tensor(s_sb[:], acp[:], bdsh[:], OP.add)
                        e_sb = hspool.tile([128, KLEN], bf16, tag="e_sb")
                        den = dens.tile([128, 1], f32, tag="den")
                        nc.scalar.activation(e_sb[:], s_sb[:], FT.Exp,
                                             scale=SCALE, accum_out=den[:])
                        rden = dens.tile([128, 1], f32, tag="rden")
                        nc.vector.reciprocal(rden[:], den[:])
                        nc.vector.tensor_scalar_mul(e_sb[:], e_sb[:], rden[:])
                        et_q = etpool.tile([128, NJ, 128], bf16, tag="et")
                        nc.scalar.dma_start_transpose(et_q[:], e_sb[:])
                        et_tiles.append(et_q)
                    avp = pav.tile([64, QLEN], f32, tag="av")
                    for qi in range(TM):
                        for js in range(NJ):
                            mm(avp[:, 128 * qi:128 * qi + 128],
                               v_sb[:, js, 64 * h:64 * h + 64],
                               et_tiles[qi][:, js, :],
                               js == 0, js == NJ - 1)
                    nc.scalar.copy(vecT[po:po + 64, t, :], avp[:])

                # ---- o-proj + residual + LN1 ----
                hsum = hpool.tile([128, KS, QLEN], bf16, tag="hh")
                hsq = hspool.tile([128, KS, QLEN], bf16, tag="hsq")
                stp = pst.tile([1, QLEN], f32, tag="st")
                st2p = pst.tile([1, QLEN], f32, tag="st")
                for m in range(KS):
                    op_ = ps.tile([128, KLEN], f32, tag="sc")
                    for k in range(KS):
                        mm(op_[:, 0:QLEN], ow[:, k, 128 * m:128 * m + 128],
                           vecT[:, k, :], k == 0, k == KS - 1)
                    nc.vector.tensor_tensor(hsum[:, m, :], op_[:, 0:QLEN],
                                            h_in[:, m, :], OP.add)
                    nc.scalar.square(hsq[:, m, :], hsum[:, m, :])
                    mm(stp[:], ones_sb[:], hsum[:, m, :], m == 0, m == KS - 1)
                    mm(st2p[:], ones_sb[:], hsq[:, m, :], m == 0, m == KS - 1)
                h_ln = _ln_tail(nc, hpool, rows, bcast, stp, st2p, hsum)

                # ---- FF ----
                z_sb = apool.tile([128, DIT, 512], bf16, tag="z_sb")
                for dt in range(DIT):
                    zp = ps.tile([128, KLEN], f32, tag="sc")
                    for k in range(KS):
                        mm(zp[:, 0:QLEN], w1[:, k, 128 * dt:128 * dt + 128],
                           h_ln[:, k, :], k == 0, k == KS - 1)
                    nc.scalar.activation(z_sb[:, dt, :], zp[:, 0:QLEN], FT.Relu,
                                         bias=b1[:, dt:dt+1], scale=1.0)
                hsum2 = hpool.tile([128, KS, QLEN], bf16, tag="hh")
                hsq2 = hspool.tile([128, KS, QLEN], bf16, tag="hsq")
                stp2 = pst.tile([1, QLEN], f32, tag="st")
                st2p2 = pst.tile([1, QLEN], f32, tag="st")
                for m in range(KS):
                    cp = ps.tile([128, KLEN], f32, tag="sc")
                    for kt in range(DIT):
                        mm(cp[:, 0:QLEN], w2[:, kt, 128 * m:128 * m + 128],
                           z_sb[:, kt, :], kt == 0, kt == DIT - 1)
                    nc.vector.tensor_tensor(hsum2[:, m, :], cp[:, 0:QLEN],
                                            h_ln[:, m, :], OP.add)
                    nc.scalar.square(hsq2[:, m, :], hsum2[:, m, :])
                    mm(stp2[:], ones_sb[:], hsum2[:, m, :], m == 0, m == KS - 1)
                    mm(st2p2[:], ones_sb[:], hsq2[:, m, :], m == 0, m == KS - 1)
                return _ln_tail(nc, hpool, rows, bcast, stp2, st2p2, hsum2)

            def body():
                h_cur = hpool.tile([128, KS, QLEN], bf16, tag="hh")
                nc.sync.dma_start(h_cur[:], h0t[:])
                for l in range(L):
                    h_cur = layer(l, h_cur)

                # ---- target logits ----
                tmp = hspool.tile([128, KS, QLEN], bf16, tag="hsq")
                nc.vector.tensor_tensor(tmp[:], h_cur[:], wtgt_sb[:], OP.mult)
                tlp = pst.tile([1, QLEN], f32, tag="st")
                for k in range(KS):
                    mm(tlp[:], ones_sb[:], tmp[:, k, :], k == 0, k == KS - 1)
                tl_sb = rows.tile([1, QLEN], f32, tag="row")
                nc.vector.tensor_copy(tl_sb[:], tlp[:])
                nc.sync.dma_start(tl_o[:], tl_sb[:])

                # ---- vocab-half log-sum-exp partial sums ----
                sums_acc = singles.tile([128, TM, NCH], f32, tag="sums_acc")
                for ch in range(NCH):
                    wv = lsep.tile([128, KS, 512], bf16, tag="wv")
                    nc.sync.dma_start(wv[:], owv_d[:, :, 512 * ch:512 * ch + 512])
                    for m in range(TM):
                        lp = ps.tile([128, KLEN], f32, tag="sc")
                        for k in range(KS):
                            mm(lp[:, 0:512], h_cur[:, k, 128 * m:128 * m + 128],
                               wv[:, k, :], k == 0, k == KS - 1)
                        e_scr = hspool.tile([128, 512], bf16, tag="bdsh")
                        nc.scalar.activation(e_scr[:], lp[:, 0:512], FT.Exp,
                                             scale=1.0,
                                             accum_out=sums_acc[:, m, ch:ch+1])
                svec = rows.tile([128, TM], f32, tag="svec")
                nc.vector.tensor_reduce(svec[:], sums_acc[:], mybir.AxisListType.X, OP.add)
                nc.sync.dma_start(sums_o[:], svec[:])

            if niter == 1:
                body()
            else:
                with tc.For_i(0, niter, 1):
                    body()

    nc.compile()
    _NC_CACHE[key] = nc
    return nc


def _ln_tail(nc, hpool, rows, bcast, stp, st2p, hsum):
    """mean/var from PSUM stat rows -> normalized bf16 tile (g==1, b==0)."""
    nm = rows.tile([1, QLEN], f32, tag="row")
    nc.vector.tensor_scalar_mul(nm[:], stp[:], -1.0 / D)      # -mean
    ms = rows.tile([1, QLEN], f32, tag="row")
    nc.vector.tensor_scalar_mul(ms[:], st2p[:], 1.0 / D)      # E[x^2]
    m2 = rows.tile([1, QLEN], f32, tag="row")
    nc.scalar.square(m2[:], nm[:])
    var = rows.tile([1, QLEN], f32, tag="row")
    nc.vector.tensor_tensor(var[:], ms[:], m2[:], OP.subtract)
    nc.vector.tensor_scalar_add(var[:], var[:], EPS)
    sd = rows.tile([1, QLEN], f32, tag="row")
    nc.scalar.sqrt(sd[:], var[:])
    rs = rows.tile([1, QLEN], f32, tag="row")
    nc.vector.reciprocal(rs[:], sd[:])
    nm_b = bcast.tile([128, QLEN], f32, tag="bcast")
    rs_b = bcast.tile([128, QLEN], f32, tag="bcast")
    nc.gpsimd.partition_broadcast(nm_b[:], nm[:])
    nc.gpsimd.partition_broadcast(rs_b[:], rs[:])
    h_ln = hpool.tile([128, KS, QLEN], bf16, tag="hh")
    for m in range(KS):
        nc.vector.tensor_tensor(h_ln[:, m, :], hsum[:, m, :], nm_b[:], OP.add)
        nc.vector.tensor_tensor(h_ln[:, m, :], h_ln[:, m, :], rs_b[:], OP.mult)
    return h_ln


def _t128(a):
    """[rows, D_contr] -> [128, D_contr//128, rows] (contraction-major tiling)."""
    d = a.shape[1]
    return np.ascontiguousarray(a.T.reshape(d // 128, 128, a.shape[0]).transpose(1, 0, 2))


def _bf(a):
    return np.asarray(a, np.float32).astype(ml_dtypes.bfloat16)


def host_prep(inp, target, mems, emb_W, out_W, out_b, r_w_bias, r_r_bias,
              qkv_W, r_W, o_W, ln1_g, ln1_b, ff_W1, ff_b1, ff_W2, ff_b2,
              ln2_g, ln2_b):
    f = np.float32
    assert np.all(ln1_g == 1) and np.all(ln2_g == 1), "LN gain != 1 unsupported"
    assert not (np.any(ln1_b) or np.any(ln2_b) or np.any(ff_b2) or np.any(out_b)), \
        "nonzero ln_b/ff_b2/out_b unsupported"

    inv_freq = 1.0 / (10000.0 ** (np.arange(0, D, 2, dtype=f) / f(D)))
    pos_seq = np.arange(KLEN - 1, -1, -1, dtype=f)
    sin_inp = pos_seq[:, None] * inv_freq[None, :]
    r = np.concatenate([np.sin(sin_inp), np.cos(sin_inp)], -1).astype(f)

    shared = {
        "qkvwt": np.stack([_t128(qkv_W[l].astype(f)) for l in range(L)]),
        "rwt": np.stack([_t128(r_W[l].astype(f)) for l in range(L)]),
        "owt": np.stack([_t128(o_W[l].astype(f)) for l in range(L)]),
        "w1t": np.stack([_t128(ff_W1[l].astype(f)) for l in range(L)]),
        "w2t": np.stack([_t128(ff_W2[l].astype(f)) for l in range(L)]),
        "rt": _t128(r),
        "rbias": np.concatenate([
            np.ascontiguousarray(r_w_bias.astype(f).reshape(KS, 128).T),
            np.ascontiguousarray(r_r_bias.astype(f).reshape(KS, 128).T)], axis=1),
        "b1t": np.stack([np.ascontiguousarray(ff_b1[l].astype(f).reshape(DIT, 128).T)
                         for l in range(L)]),
    }
    for k in ("qkvwt", "rwt", "owt", "w1t", "w2t", "rt"):
        shared[k] = shared[k].astype(ml_dtypes.bfloat16)

    h0 = emb_W[np.asarray(inp)].astype(f) * f(D ** 0.5)        # [q, b, D]
    owv_halves = []
    for vh in range(2):
        lo, hi = (0, VSPLIT) if vh == 0 else (VSPLIT, V)
        wc = np.zeros((VH, D), f)
        wc[:hi - lo] = out_W[lo:hi].astype(f)
        owv_halves.append(_t128(wc).astype(ml_dtypes.bfloat16))

    in_maps = []
    for c in range(NCORES):
        b, vh = c % BSZ, c // BSZ
        rows_c = np.arange(QLEN) * BSZ + b
        m = dict(shared)
        m["h0t"] = _bf(_t128(h0[:, b, :]))
        m["memst"] = np.stack([_t128(mems[l, :, b, :].astype(f)) for l in range(L)]
                              ).astype(ml_dtypes.bfloat16)
        m["wtgt"] = _bf(_t128(out_W[np.asarray(target)[rows_c]].astype(f)))
        m["owv"] = owv_halves[vh]
        in_maps.append(m)
    return in_maps


def kernel(inp, target, mems, emb_W, out_W, out_b, r_w_bias, r_r_bias,
           qkv_W, r_W, o_W, ln1_g, ln1_b, ff_W1, ff_b1, ff_W2, ff_b2,
           ln2_g, ln2_b):
    global LAST_RESULTS
    args = [np.asarray(a) for a in (inp, target, mems, emb_W, out_W, out_b,
                                    r_w_bias, r_r_bias, qkv_W, r_W, o_W,
                                    ln1_g, ln1_b, ff_W1, ff_b1, ff_W2, ff_b2,
                                    ln2_g, ln2_b)]
    in_maps = host_prep(*args)
    nc = build_nc(1)
    res = run_bass_kernel_spmd(nc, in_maps, list(range(NCORES)))
    LAST_RESULTS = res
    target = np.asarray(target)
    out_b = np.asarray(out_b, np.float32)

    nll = np.zeros(QLEN * BSZ, np.float64)
    for b in range(BSZ):
        s0 = np.asarray(res.results[b]["sums"], np.float64)        # [128, TM]
        s1 = np.asarray(res.results[b + 4]["sums"], np.float64)
        S = (s0.T.reshape(-1) - PADS[0]) + (s1.T.reshape(-1) - PADS[1])  # [512] per q
        tl = np.asarray(res.results[b]["tl"], np.float64).reshape(-1)    # [512]
        rows_c = np.arange(QLEN) * BSZ + b
        nll[rows_c] = np.log(S) - (tl + out_b[target[rows_c]])
    return nll.astype(np.float32)


# revision 31
# speedup vs baseline: 2.2605x; 2.2605x over previous
import sys

for _p in ("/opt/trn_rl_repo",):
    if _p not in sys.path:
        sys.path.insert(0, _p)

import numpy as np
import ml_dtypes

import concourse.bass as bass
import concourse.mybir as mybir
import concourse.tile as tile
from concourse import bacc
from concourse.bass_types import AP
from concourse.bass_utils import run_bass_kernel_spmd

V, L, H, DH, D, DI = 50257, 6, 8, 64, 512, 2048
QLEN, MLEN, BSZ = 512, 512, 4
KLEN = QLEN + MLEN
NCORES = 8
KS = D // 128            # 4 contraction sub-tiles
DIT = DI // 128          # 16
TM = QLEN // 128         # 4 query/token tiles
NJ = KLEN // 128         # 8 key sub-tiles
VSPLIT = 25128           # vocab half boundary: [0,25128) / [25128,50257)
VH = 25600               # padded per-half vocab (50 chunks of 512)
NCH = VH // 512          # 50
PADS = (VH - VSPLIT, VH - (V - VSPLIT))   # exp(0)=1 contributions to subtract
SCALE = 1.0 / (DH ** 0.5)
MASKVAL = -30000.0
EPS = 1e-5

bf16 = mybir.dt.bfloat16
f32 = mybir.dt.float32
FT = mybir.ActivationFunctionType
OP = mybir.AluOpType

_NC_CACHE = {}
LAST_RESULTS = None


def build_nc(niter=1, sim=False, nlayers=L, do_lse=True):
    key = ("nc", niter, sim, nlayers, do_lse)
    if key in _NC_CACHE:
        return _NC_CACHE[key]
    # Bacc (not plain Bass): its compile() runs generate_event_semaphores,
    # which splits multi-sem waits to satisfy the 1-wait-per-instruction HW
    # constraint.
    nc = bacc.Bacc("TRN2", target_bir_lowering=False, debug=sim)
    h0t = nc.dram_tensor("h0t", [128, KS, QLEN], bf16, kind="ExternalInput")
    memst = nc.dram_tensor("memst", [L, 128, KS, MLEN], bf16, kind="ExternalInput")
    qkvwt = nc.dram_tensor("qkvwt", [L, 128, KS, 3 * H * DH], bf16, kind="ExternalInput")
    rwt = nc.dram_tensor("rwt", [L, 128, KS, H * DH], bf16, kind="ExternalInput")
    owt = nc.dram_tensor("owt", [L, 128, KS, D], bf16, kind="ExternalInput")
    w1t = nc.dram_tensor("w1t", [L, 128, KS, DI], bf16, kind="ExternalInput")
    w2t = nc.dram_tensor("w2t", [L, 128, DIT, D], bf16, kind="ExternalInput")
    rt_d = nc.dram_tensor("rt", [128, KS, KLEN], bf16, kind="ExternalInput")
    rbias_d = nc.dram_tensor("rbias", [128, 2 * KS], f32, kind="ExternalInput")
    b1t = nc.dram_tensor("b1t", [L, 128, DIT], f32, kind="ExternalInput")
    wtgt_d = nc.dram_tensor("wtgt", [128, KS, QLEN], bf16, kind="ExternalInput")
    owv_d = nc.dram_tensor("owv", [128, KS, VH], bf16, kind="ExternalInput")
    sums_o = nc.dram_tensor("sums", [128, TM], f32, kind="ExternalOutput")
    tl_o = nc.dram_tensor("tl", [1, QLEN], f32, kind="ExternalOutput")

    NBD = 3
    with tile.TileContext(nc) as tc:
        with (
            tc.tile_pool(name="singles", bufs=1) as singles,
            tc.tile_pool(name="wq", bufs=1) as wq,
            tc.tile_pool(name="wf", bufs=1) as wf,
            tc.tile_pool(name="act", bufs=1) as apool,
            tc.tile_pool(name="hh", bufs=3) as hpool,
            tc.tile_pool(name="hs", bufs=1) as hspool,
            tc.tile_pool(name="attn", bufs=3) as attnp,
            tc.tile_pool(name="et", bufs=2) as etpool,
            tc.tile_pool(name="rows", bufs=8) as rows,
            tc.tile_pool(name="bcast", bufs=2) as bcast,
            tc.tile_pool(name="dens", bufs=8) as dens,
            tc.tile_pool(name="lse", bufs=2) as lsep,
            tc.tile_pool(name="ps", bufs=7, space="PSUM") as ps,
            tc.tile_pool(name="pav", bufs=1, space="PSUM") as pav,
        ):
            # loop-invariant loads.  The HW allows only ~3 semaphore waits
            # per instruction, so absorb the DMA-lane waits into one sync-
            # engine NOP (3 waits), run the memsets after it, then raise a
            # barrier whose own wait set is tiny.  Downstream instructions
            # then wait on the barrier only.
            from concourse.tile_rust import add_dep_helper
            rt_sb = singles.tile([128, KS, KLEN], bf16, tag="rt")
            d1 = nc.sync.dma_start(rt_sb[:], rt_d[:])
            rbias = singles.tile([128, 2 * KS], f32, tag="rbias")
            d2 = nc.sync.dma_start(rbias[:], rbias_d[:])
            rwb = rbias[:, 0:KS]
            rrb = rbias[:, KS:2 * KS]
            wtgt_sb = singles.tile([128, KS, QLEN], bf16, tag="wtgt")
            d3 = nc.sync.dma_start(wtgt_sb[:], wtgt_d[:])
            n1 = nc.sync.nop()
            for d_ in (d1, d2):
                add_dep_helper(n1.ins, d_.ins, sync=True, reason="absorb dma lanes")
            n2 = nc.sync.nop()
            add_dep_helper(n2.ins, d3.ins, sync=True, reason="absorb dma lanes")
            ones_sb = singles.tile([128, 1], bf16, tag="ones")
            nc.gpsimd.memset(ones_sb[:], 1.0)
            # rel-shift shear sources: upper halves hold the causal-mask fill
            # so the shear read applies shift+mask in one DMA.  Allocated once
            # (persistent tiles) so the fill survives all iterations.
            bd_raws = [singles.tile([128, 2 * KLEN], bf16, tag=f"bdraw{i}", name=f"bdraw{i}")
                       for i in range(NBD)]
            for t_ in bd_raws:
                nc.gpsimd.memset(t_[:, KLEN:], MASKVAL)
            tc.strict_bb_all_engine_barrier()

            bd_state = [0]

            def mm(out, lhsT, rhs, start, stop):
                nc.tensor.matmul(out, lhsT, rhs, start=start, stop=stop)

            def layer(l, h_in):
                qkvw = wq.tile([128, KS, 3 * H * DH], bf16, tag="qkvw")
                nc.sync.dma_start(qkvw[:], qkvwt[l])
                rw = wq.tile([128, KS, H * DH], bf16, tag="rw")
                nc.sync.dma_start(rw[:], rwt[l])
                ow = wq.tile([128, KS, D], bf16, tag="ow")
                nc.sync.dma_start(ow[:], owt[l])
                w1 = wf.tile([128, KS, DI], bf16, tag="w1")
                nc.sync.dma_start(w1[:], w1t[l])
                w2 = wf.tile([128, DIT, D], bf16, tag="w2")
                nc.sync.dma_start(w2[:], w2t[l])
                b1 = dens.tile([128, DIT], f32, tag="b1")
                nc.sync.dma_start(b1[:], b1t[l])

                mem_sb = apool.tile([128, KS, MLEN], bf16, tag="mem_sb")
                nc.sync.dma_start(mem_sb[:], memst[l])

                def cat_slice(k, lo, hi):
                    # concatenated sequence [mems; h]: tokens [0,512) from
                    # mems, [512,1024) from the residual stream
                    if hi <= MLEN:
                        return mem_sb[:, k, lo:hi]
                    assert lo >= MLEN
                    return h_in[:, k, lo - MLEN:hi - MLEN]

                # ---- projections ----
                q_rw = apool.tile([128, KS, QLEN], bf16, tag="q_rw")
                q_rr = apool.tile([128, KS, QLEN], bf16, tag="q_rr")
                for m in range(KS):  # q feature tiles
                    qp = ps.tile([128, 512], f32, tag="sc")
                    for k in range(KS):
                        mm(qp[:], qkvw[:, k, 128 * m:128 * m + 128],
                           h_in[:, k, :], k == 0, k == KS - 1)
                    nc.scalar.activation(q_rw[:, m, :], qp[:], FT.Identity,
                                         bias=rwb[:, m:m+1], scale=1.0)
                    nc.scalar.activation(q_rr[:, m, :], qp[:], FT.Identity,
                                         bias=rrb[:, m:m+1], scale=1.0)

                k_sb = apool.tile([128, KS, KLEN], bf16, tag="k_sb")
                for m in range(KS):  # k feature tiles
                    for half in range(2):
                        kp = ps.tile([128, 512], f32, tag="sc")
                        for k in range(KS):
                            mm(kp[:],
                               qkvw[:, k, 512 + 128 * m:512 + 128 * m + 128],
                               cat_slice(k, 512 * half, 512 * half + 512),
                               k == 0, k == KS - 1)
                        nc.scalar.copy(k_sb[:, m, 512 * half:512 * half + 512], kp[:])

                v_sb = apool.tile([128, NJ, 512], bf16, tag="v_sb")  # token-major
                for js in range(NJ):
                    vp = ps.tile([128, 512], f32, tag="sc")
                    for k in range(KS):
                        mm(vp[:], cat_slice(k, 128 * js, 128 * js + 128),
                           qkvw[:, k, 1024:1536], k == 0, k == KS - 1)
                    nc.scalar.copy(v_sb[:, js, :], vp[:])

                rk_sb = apool.tile([128, KS, KLEN], bf16, tag="rk_sb")
                for m in range(KS):
                    for half in range(2):
                        rp = ps.tile([128, 512], f32, tag="sc")
                        for k in range(KS):
                            mm(rp[:],
                               rw[:, k, 128 * m:128 * m + 128],
                               rt_sb[:, k, 512 * half:512 * half + 512],
                               k == 0, k == KS - 1)
                        nc.scalar.copy(rk_sb[:, m, 512 * half:512 * half + 512], rp[:])

                # ---- attention ----
                vecT = apool.tile([128, KS, QLEN], bf16, tag="vecT")

                def av(h, et_full):
                    t = h // 2
                    po = 64 * (h % 2)
                    avp = pav.tile([64, QLEN], f32, tag="av")
                    for js in range(NJ):
                        mm(avp[:], v_sb[:, js, 64 * h:64 * h + 64],
                           et_full[:, js, :], js == 0, js == NJ - 1)
                    nc.scalar.copy(vecT[po:po + 64, t, :], avp[:])

                prev = None
                for h in range(H):
                    t = h // 2
                    po = 64 * (h % 2)
                    et_full = etpool.tile([128, NJ, QLEN], bf16, tag="et")
                    for qi in range(TM):
                        bd = bd_raws[bd_state[0] % NBD]
                        bd_state[0] += 1
                        for half in range(2):
                            bdp = ps.tile([128, 512], f32, tag="sc")
                            mm(bdp[:],
                               q_rr[po:po + 64, t, 128 * qi:128 * qi + 128],
                               rk_sb[po:po + 64, t, 512 * half:512 * half + 512],
                               True, True)
                            nc.vector.tensor_copy(
                                bd[:, 512 * half:512 * half + 512], bdp[:])
                        bdsh = attnp.tile([128, KLEN], bf16, tag="bdsh")
                        bap = bd[:]
                        shear = AP(bap.tensor, bap.offset + 511 - 128 * qi,
                                   [[2 * KLEN - 1, 128], [1, KLEN]])
                        nc.sync.dma_start(bdsh[:], shear)
                        s_sb = attnp.tile([128, KLEN], bf16, tag="s_sb")
                        for half in range(2):
                            acp = ps.tile([128, 512], f32, tag="sc")
                            mm(acp[:],
                               q_rw[po:po + 64, t, 128 * qi:128 * qi + 128],
                               k_sb[po:po + 64, t, 512 * half:512 * half + 512],
                               True, True)
                            nc.vector.tensor_tensor(
                                s_sb[:, 512 * half:512 * half + 512], acp[:],
                                bdsh[:, 512 * half:512 * half + 512], OP.add)
                        e_sb = attnp.tile([128, KLEN], bf16, tag="e_sb")
                        den = dens.tile([128, 1], f32, tag="den")
                        nc.scalar.activation(e_sb[:], s_sb[:], FT.Exp,
                                             scale=SCALE, accum_out=den[:])
                        rden = dens.tile([128, 1], f32, tag="rden")
                        nc.vector.reciprocal(rden[:], den[:])
                        nc.vector.tensor_scalar_mul(e_sb[:], e_sb[:], rden[:])
                        nc.scalar.dma_start_transpose(
                            et_full[:, :, 128 * qi:128 * qi + 128], e_sb[:])
                    if prev is not None:
                        av(*prev)
                    prev = (h, et_full)
                av(*prev)

                # ---- o-proj + residual + LN1 ----
                hsum = hpool.tile([128, KS, QLEN], bf16, tag="hh")
                hsq = hspool.tile([128, KS, QLEN], bf16, tag="hsq")
                stp = ps.tile([1, QLEN], f32, tag="sc", name="stp")
                st2p = ps.tile([1, QLEN], f32, tag="sc", name="st2p")
                for m in range(KS):
                    op_ = ps.tile([128, 512], f32, tag="sc")
                    for k in range(KS):
                        mm(op_[:], ow[:, k, 128 * m:128 * m + 128],
                           vecT[:, k, :], k == 0, k == KS - 1)
                    nc.vector.tensor_tensor(hsum[:, m, :], op_[:],
                                            h_in[:, m, :], OP.add)
                    nc.scalar.square(hsq[:, m, :], hsum[:, m, :])
                    mm(stp[:], ones_sb[:], hsum[:, m, :], m == 0, m == KS - 1)
                    mm(st2p[:], ones_sb[:], hsq[:, m, :], m == 0, m == KS - 1)
                h_ln = _ln_tail(nc, hpool, rows, bcast, stp, st2p, hsum)

                # ---- FF ----  (z in di-halves; c psums accumulate across)
                cps = [ps.tile([128, 512], f32, tag="sc", name=f"cp{l}_{m}")
                       for m in range(KS)]
                for hf in range(2):
                    z_sb = apool.tile([128, DIT // 2, 512], bf16, tag="z_sb")
                    for dt in range(DIT // 2):
                        dtg = hf * (DIT // 2) + dt
                        zp = ps.tile([128, 512], f32, tag="sc")
                        for k in range(KS):
                            mm(zp[:], w1[:, k, 128 * dtg:128 * dtg + 128],
                               h_ln[:, k, :], k == 0, k == KS - 1)
                        nc.scalar.activation(z_sb[:, dt, :], zp[:], FT.Relu,
                                             bias=b1[:, dtg:dtg+1], scale=1.0)
                    for m in range(KS):
                        for kt in range(DIT // 2):
                            ktg = hf * (DIT // 2) + kt
                            mm(cps[m][:], w2[:, ktg, 128 * m:128 * m + 128],
                               z_sb[:, kt, :], ktg == 0, ktg == DIT - 1)
                hsum2 = hpool.tile([128, KS, QLEN], bf16, tag="hh")
                hsq2 = hspool.tile([128, KS, QLEN], bf16, tag="hsq")
                stp2 = ps.tile([1, QLEN], f32, tag="sc", name="stp2")
                st2p2 = ps.tile([1, QLEN], f32, tag="sc", name="st2p2")
                for m in range(KS):
                    nc.vector.tensor_tensor(hsum2[:, m, :], cps[m][:],
                                            h_ln[:, m, :], OP.add)
                    nc.scalar.square(hsq2[:, m, :], hsum2[:, m, :])
                    mm(stp2[:], ones_sb[:], hsum2[:, m, :], m == 0, m == KS - 1)
                    mm(st2p2[:], ones_sb[:], hsq2[:, m, :], m == 0, m == KS - 1)
                return _ln_tail(nc, hpool, rows, bcast, stp2, st2p2, hsum2)

            def body():
                h_cur = hpool.tile([128, KS, QLEN], bf16, tag="hh")
                nc.sync.dma_start(h_cur[:], h0t[:])
                for l in range(nlayers):
                    h_cur = layer(l, h_cur)

                # ---- target logits ----
                tmp = hspool.tile([128, KS, QLEN], bf16, tag="hsq")
                nc.vector.tensor_tensor(tmp[:], h_cur[:], wtgt_sb[:], OP.mult)
                tlp = ps.tile([1, QLEN], f32, tag="sc", name="tlp")
                for k in range(KS):
                    mm(tlp[:], ones_sb[:], tmp[:, k, :], k == 0, k == KS - 1)
                tl_sb = rows.tile([1, QLEN], f32, tag="row")
                nc.vector.tensor_copy(tl_sb[:], tlp[:])
                nc.sync.dma_start(tl_o[:], tl_sb[:])

                # ---- vocab-half log-sum-exp partial sums ----
                sums_acc = singles.tile([128, TM, NCH], f32, tag="sums_acc")
                nch = NCH if do_lse else 1
                for ch in range(nch):
                    wv = lsep.tile([128, KS, 512], bf16, tag="wv")
                    nc.sync.dma_start(wv[:], owv_d[:, :, 512 * ch:512 * ch + 512])
                    for m in range(TM):
                        lp = ps.tile([128, 512], f32, tag="sc")
                        for k in range(KS):
                            mm(lp[:], h_cur[:, k, 128 * m:128 * m + 128],
                               wv[:, k, :], k == 0, k == KS - 1)
                        e_scr = attnp.tile([128, 512], bf16, tag="bdsh")
                        nc.scalar.activation(e_scr[:], lp[:], FT.Exp,
                                             scale=1.0,
                                             accum_out=sums_acc[:, m, ch:ch+1])
                svec = rows.tile([128, TM], f32, tag="svec")
                nc.vector.tensor_reduce(svec[:], sums_acc[:], mybir.AxisListType.X, OP.add)
                nc.sync.dma_start(sums_o[:], svec[:])

            if niter == 1:
                body()
            else:
                with tc.For_i(0, niter, 1):
                    body()

    nc.compile()
    _NC_CACHE[key] = nc
    return nc


def _ln_tail(nc, hpool, rows, bcast, stp, st2p, hsum):
    """mean/var from PSUM stat rows -> normalized bf16 tile (g==1, b==0)."""
    nm = rows.tile([1, QLEN], f32, tag="row")
    nc.vector.tensor_scalar_mul(nm[:], stp[:], -1.0 / D)      # -mean
    ms = rows.tile([1, QLEN], f32, tag="row")
    nc.vector.tensor_scalar_mul(ms[:], st2p[:], 1.0 / D)      # E[x^2]
    m2 = rows.tile([1, QLEN], f32, tag="row")
    nc.scalar.square(m2[:], nm[:])
    var = rows.tile([1, QLEN], f32, tag="row")
    nc.vector.tensor_tensor(var[:], ms[:], m2[:], OP.subtract)
    nc.vector.tensor_scalar_add(var[:], var[:], EPS)
    sd = rows.tile([1, QLEN], f32, tag="row")
    nc.scalar.sqrt(sd[:], var[:])
    rs = rows.tile([1, QLEN], f32, tag="row")
    nc.vector.reciprocal(rs[:], sd[:])
    nm_b = bcast.tile([128, QLEN], f32, tag="bcast")
    rs_b = bcast.tile([128, QLEN], f32, tag="bcast")
    nc.gpsimd.partition_broadcast(nm_b[:], nm[:])
    nc.gpsimd.partition_broadcast(rs_b[:], rs[:])
    h_ln = hpool.tile([128, KS, QLEN], bf16, tag="hh")
    for m in range(KS):
        nc.vector.tensor_tensor(h_ln[:, m, :], hsum[:, m, :], nm_b[:], OP.add)
        nc.vector.tensor_tensor(h_ln[:, m, :], h_ln[:, m, :], rs_b[:], OP.mult)
    return h_ln


def _t128(a):
    """[rows, D_contr] -> [128, D_contr//128, rows] (contraction-major tiling)."""
    d = a.shape[1]
    return np.ascontiguousarray(a.T.reshape(d // 128, 128, a.shape[0]).transpose(1, 0, 2))


def _bf(a):
    return np.asarray(a, np.float32).astype(ml_dtypes.bfloat16)


def host_prep(inp, target, mems, emb_W, out_W, out_b, r_w_bias, r_r_bias,
              qkv_W, r_W, o_W, ln1_g, ln1_b, ff_W1, ff_b1, ff_W2, ff_b2,
              ln2_g, ln2_b):
    f = np.float32
    assert np.all(ln1_g == 1) and np.all(ln2_g == 1), "LN gain != 1 unsupported"
    assert not (np.any(ln1_b) or np.any(ln2_b) or np.any(ff_b2) or np.any(out_b)), \
        "nonzero ln_b/ff_b2/out_b unsupported"

    inv_freq = 1.0 / (10000.0 ** (np.arange(0, D, 2, dtype=f) / f(D)))
    pos_seq = np.arange(KLEN - 1, -1, -1, dtype=f)
    sin_inp = pos_seq[:, None] * inv_freq[None, :]
    r = np.concatenate([np.sin(sin_inp), np.cos(sin_inp)], -1).astype(f)

    shared = {
        "qkvwt": np.stack([_t128(qkv_W[l].astype(f)) for l in range(L)]),
        "rwt": np.stack([_t128(r_W[l].astype(f)) for l in range(L)]),
        "owt": np.stack([_t128(o_W[l].astype(f)) for l in range(L)]),
        "w1t": np.stack([_t128(ff_W1[l].astype(f)) for l in range(L)]),
        "w2t": np.stack([_t128(ff_W2[l].astype(f)) for l in range(L)]),
        "rt": _t128(r),
        "rbias": np.concatenate([
            np.ascontiguousarray(r_w_bias.astype(f).reshape(KS, 128).T),
            np.ascontiguousarray(r_r_bias.astype(f).reshape(KS, 128).T)], axis=1),
        "b1t": np.stack([np.ascontiguousarray(ff_b1[l].astype(f).reshape(DIT, 128).T)
                         for l in range(L)]),
    }
    for k in ("qkvwt", "rwt", "owt", "w1t", "w2t", "rt"):
        shared[k] = shared[k].astype(ml_dtypes.bfloat16)

    h0 = emb_W[np.asarray(inp)].astype(f) * f(D ** 0.5)        # [q, b, D]
    owv_halves = []
    for vh in range(2):
        lo, hi = (0, VSPLIT) if vh == 0 else (VSPLIT, V)
        wc = np.zeros((VH, D), f)
        wc[:hi - lo] = out_W[lo:hi].astype(f)
        owv_halves.append(_t128(wc).astype(ml_dtypes.bfloat16))

    in_maps = []
    for c in range(NCORES):
        b, vh = c % BSZ, c // BSZ
        rows_c = np.arange(QLEN) * BSZ + b
        m = dict(shared)
        m["h0t"] = _bf(_t128(h0[:, b, :]))
        m["memst"] = np.stack([_t128(mems[l, :, b, :].astype(f)) for l in range(L)]
                              ).astype(ml_dtypes.bfloat16)
        m["wtgt"] = _bf(_t128(out_W[np.asarray(target)[rows_c]].astype(f)))
        m["owv"] = owv_halves[vh]
        in_maps.append(m)
    return in_maps


def kernel(inp, target, mems, emb_W, out_W, out_b, r_w_bias, r_r_bias,
           qkv_W, r_W, o_W, ln1_g, ln1_b, ff_W1, ff_b1, ff_W2, ff_b2,
           ln2_g, ln2_b):
    global LAST_RESULTS
    args = [np.asarray(a) for a in (inp, target, mems, emb_W, out_W, out_b,
                                    r_w_bias, r_r_bias, qkv_W, r_W, o_W,
                                    ln1_g, ln1_b, ff_W1, ff_b1, ff_W2, ff_b2,
                                    ln2_g, ln2_b)]
    in_maps = host_prep(*args)
    nc = build_nc(1)
    res = run_bass_kernel_spmd(nc, in_maps, list(range(NCORES)))
    LAST_RESULTS = res
    target = np.asarray(target)
    out_b = np.asarray(out_b, np.float32)

    nll = np.zeros(QLEN * BSZ, np.float64)
    for b in range(BSZ):
        s0 = np.asarray(res.results[b]["sums"], np.float64)        # [128, TM]
        s1 = np.asarray(res.results[b + 4]["sums"], np.float64)
        S = (s0.T.reshape(-1) - PADS[0]) + (s1.T.reshape(-1) - PADS[1])  # [512] per q
        tl = np.asarray(res.results[b]["tl"], np.float64).reshape(-1)    # [512]
        rows_c = np.arange(QLEN) * BSZ + b
        nll[rows_c] = np.log(S) - (tl + out_b[target[rows_c]])
    return nll.astype(np.float32)
